# revision 7
# baseline (speedup 1.0000x reference)
"""Trainium2 Bass kernel for nn_CausalUnlabeled_2044404433206 (moe_routing).

Model per sample:
  e    = emb[f, x_cate[:, f]]                 (16 fields x 8 dims = 128 feats)
  x    = concat(x_cont[64], e[128])           -> 192
  h1   = relu(x @ W1 + b1)                    -> 32
  h2   = relu(h1 @ W2 + b2)                   -> 32
  r    = h2 @ W3 + b3                         -> 32
  hh   = relu(r @ HW1[t] + Hb1[t])            -> 16   (only the routed head)
  y    = hh @ HW2[t] + Hb2[t]

Design (v4):
  * Samples GLOBALLY SORTED by routing head t on the host (input
    marshalling, like the embedding gather), sharded contiguously across
    8 cores, padded so every 512-sample lane is single-headed.  Each
    lane uses only its own head's weights (baked into per-tile weight
    DATA m3all/wh2p since lhsT offsets are static), so head layers do
    1/8 the work and no one-hot select machinery is needed.
  * L3 fused into H1: r @ HW1[n] = h2 @ (W3 @ HW1[n]) = h2 @ M3[n],
    M3 precomputed host-side.  One less matmul stage + PSUM eviction.
  * Embedding features ship as fp8e4m3 (e-values ~0.05 meet e-weights
    ~0.05: fp8's ~2% element error lands at ~0.2% of h1).
  * DMA floor is ~2.9us per tile-pair (1MB @ ~358GB/s per-core HBM);
    everything else is structured to stay below it:
    - evictions run at FD=1024 over tile-PAIR psum tiles (p1p/p2p span
      2 banks; each tile's matmuls stay within one bank), split
      ACT: h1p+hh+y vs DVE: h2p -- each DVE op pays a full pipe DRAIN,
      so minimizing DVE op count matters more than balancing elements;
    - tile pairs share one H1' bank (even tile rows 32j..+16 via
      [M3|0] lhsT blocks, odd rows +16..+32 via [0|M3]), one hh
      eviction and ONE merged H2 matmul per pair (disjoint lhsT rows
      and output rows encode both tiles in one [128,16] block);
    - H2 accumulates a group of 4 tiles into one [16, L] bank.
  * PE HAM warm-up: the clock gate only opens (1.2 -> 2.4 GHz) after a
    ~3.4us continuously-busy window; a 10-matmul back-to-back dummy
    burst at kernel start provides it while the first chunks stream in.
    Steady-state PE gaps are far below the ~3.4us MID window, so it
    never re-throttles mid-run.
  * Inputs stream in 4-tile 1MB chunks, 2 chunks prefetched ahead; the
    xc chunk is split into partition halves 0:64 / 64:128 (even / odd
    SBUF ports) so all 16 DMA engines carry it (W1c duplicated into
    both halves; second-half L1 uses tile_position=(64, 32j)).
  * y stores stream out per group; output is f16 (y ~1e-3 scale, f16
    rounding ~5e-4 relative -- far under the 2e-2 gate).
"""

import os
import sys

sys.path.insert(0, "/opt/trn_rl_repo")

import numpy as np

B_FULL = 524288
CONT = 64
NF = 16  # categorical fields
VOCAB = 1000
EM = 8
LOW = EM * NF + CONT  # 192
RH = 32
RR = 32  # representation dim
PH = 16
NH = 8
N_CORES = 8
T = 2048  # samples per device tile
LANES = 4
L = T // LANES  # 512
GRP = 4  # tiles per H2 accumulation group / DMA chunk
NWARM = 10  # PE warm-up matmuls

_NC_CACHE = {}


def _build(nt, nobias=False):
    """Build + compile the per-core Bass program for nt tiles of T samples."""
    from contextlib import ExitStack

    import concourse.mybir as mybir
    import concourse.tile as tile
    from concourse import bacc

    f32 = mybir.dt.float32
    f16 = mybir.dt.float16
    f8 = mybir.dt.float8e4
    AF = mybir.ActivationFunctionType
    OP = mybir.AluOpType

    bs = nt * T
    ngroups = (nt + GRP - 1) // GRP
    npairs = (nt + 1) // 2

    nc = bacc.Bacc(
        "TRN2",
        target_bir_lowering=False,
        debug=False,
        enable_asserts=False,
        num_devices=N_CORES,
    )

    # ---- DRAM I/O ----
    d_xcT = nc.dram_tensor("xcT", [CONT, bs], f16, kind="ExternalInput")
    d_e8 = nc.dram_tensor("e8", [128, bs], f8, kind="ExternalInput")
    d_w1e = nc.dram_tensor("w1e8", [128, RH], f8, kind="ExternalInput")
    d_w1c = nc.dram_tensor("w1cd", [128, RH], f16, kind="ExternalInput")
    d_w2bd = nc.dram_tensor("w2bd", [128, 128], f16, kind="ExternalInput")
    d_m3 = nc.dram_tensor("m3all", [128, 32 * nt], f16, kind="ExternalInput")
    d_wh2 = nc.dram_tensor("wh2p", [128, 16 * npairs], f16, kind="ExternalInput")
    d_y = nc.dram_tensor("y", [16, ngroups * L], f16, kind="ExternalOutput")
    if not nobias:
        d_b1 = nc.dram_tensor("b1r", [128, 1], f32, kind="ExternalInput")
        d_b2 = nc.dram_tensor("b2r", [128, 1], f32, kind="ExternalInput")
        d_hb3 = nc.dram_tensor("hb3", [128, npairs], f32, kind="ExternalInput")
        d_hb2 = nc.dram_tensor("hb2", [16, ngroups], f32, kind="ExternalInput")

    with tile.TileContext(nc) as tc, ExitStack() as ctx:
        cpool = ctx.enter_context(tc.tile_pool(name="const", bufs=1))
        inpool = ctx.enter_context(tc.tile_pool(name="inp", bufs=3))
        apool = ctx.enter_context(tc.tile_pool(name="acts", bufs=3))
        ypool = ctx.enter_context(tc.tile_pool(name="yout", bufs=2))
        ppool = ctx.enter_context(tc.tile_pool(name="psum", bufs=1, space="PSUM"))

        def cload(dram, shape, dtype, tag):
            tl = cpool.tile(shape, dtype, tag=tag, name=tag)
            nc.sync.dma_start(tl[:], dram.ap())
            return tl

        w1e = cload(d_w1e, [128, RH], f8, "w1e")
        w1cd = cload(d_w1c, [128, RH], f16, "w1cd")
        w2bd = cload(d_w2bd, [128, 128], f16, "w2bd")
        m3 = cload(d_m3, [128, 32 * nt], f16, "m3")
        wh2 = cload(d_wh2, [128, 16 * npairs], f16, "wh2")
        if not nobias:
            b1r = cload(d_b1, [128, 1], f32, "b1r")
            b2r = cload(d_b2, [128, 1], f32, "b2r")
            hb3 = cload(d_hb3, [128, npairs], f32, "hb3")
            hb2 = cload(d_hb2, [16, ngroups], f32, "hb2")
        zeros = cpool.tile([128, 2 * L], f16, tag="zeros", name="zeros")
        nc.vector.memset(zeros[:], 0.0)

        # ---- input chunk loader: 1MB per group, 2 groups prefetched ----
        def load_chunk(G):
            glen = min(GRP, nt - G * GRP)
            clen = glen * T
            c0 = G * GRP * T
            half = (glen + 1) // 2
            xcq = inpool.tile([128, GRP * T // 2], f16, tag="xcq", name="xcq")
            nc.sync.dma_start(
                xcq[0:CONT, : half * T], d_xcT.ap()[:, c0 : c0 + half * T]
            )
            if glen > half:
                nc.sync.dma_start(
                    xcq[CONT:128, : (glen - half) * T],
                    d_xcT.ap()[:, c0 + half * T : c0 + clen],
                )
            e8q = inpool.tile([128, GRP * T], f8, tag="e8q", name="e8q")
            nc.sync.dma_start(e8q[:, :clen], d_e8.ap()[:, c0 : c0 + clen])
            return xcq, e8q, half

        chunks = {0: load_chunk(0)}
        if ngroups > 1:
            chunks[1] = load_chunk(1)

        # ---- PE warm-up: back-to-back matmuls flip HAM to 2.4 GHz while
        # the first chunks stream in.  M=16 keeps the LDWEIGHTS tiny.
        wpy = ppool.tile([16, L], f32, tag="py", bufs=1, name="wpy")
        for _ in range(NWARM):
            nc.tensor.matmul(
                wpy[:], zeros[:, :16], zeros[:, :L], start=True, stop=True,
                skip_group_check=True,
            )

        for P in range(npairs):
            i0 = 2 * P
            pglen = min(2, nt - i0)  # tiles in this pair (1 for odd tail)
            G = i0 // GRP
            glen = min(GRP, nt - G * GRP)

            if i0 % GRP == 0 and G + 2 < ngroups:
                chunks[G + 2] = load_chunk(G + 2)
            xcq, e8q, half = chunks[G]

            # ---- L1 + L2 per tile; pair shares 2-bank psum tiles ----
            p1p = ppool.tile([128, 2 * L], f32, tag="p1p", bufs=2, name="p1p")
            p2p = ppool.tile([128, 2 * L], f32, tag="p2p", bufs=1, name="p2p")
            for u in range(pglen):
                i = i0 + u
                g = i % GRP
                if g < half:
                    xrow, xcol = 0, g * T
                else:
                    xrow, xcol = CONT, (g - half) * T
                for j in range(LANES):
                    nc.tensor.matmul(
                        p1p[32 * j : 32 * j + 32, u * L : (u + 1) * L],
                        w1e[:],
                        e8q[:, (g * LANES + j) * L : (g * LANES + j + 1) * L],
                        start=True, stop=False, tile_position=(0, 32 * j),
                        skip_group_check=True,
                    )
                for j in range(LANES):
                    nc.tensor.matmul(
                        p1p[32 * j : 32 * j + 32, u * L : (u + 1) * L],
                        w1cd[xrow : xrow + CONT, :],
                        xcq[xrow : xrow + CONT, xcol + j * L : xcol + (j + 1) * L],
                        start=False, stop=True, tile_position=(xrow, 32 * j),
                        skip_group_check=True,
                    )
            h1p = apool.tile([128, 2 * L], f16, tag="h1p", name="h1p")
            if nobias:
                nc.scalar.activation(h1p[:], p1p[:], AF.Relu)
            else:
                nc.scalar.activation(h1p[:], p1p[:], AF.Relu, bias=b1r[:])

            for u in range(pglen):
                nc.tensor.matmul(
                    p2p[:, u * L : (u + 1) * L], w2bd[:],
                    h1p[:, u * L : (u + 1) * L],
                    start=True, stop=True, skip_group_check=True,
                )
            h2p = apool.tile([128, 2 * L], f16, tag="h2p", name="h2p")
            if nobias:
                nc.vector.tensor_scalar_max(h2p[:], p2p[:], 0.0)
            else:
                nc.vector.scalar_tensor_tensor(
                    h2p[:], p2p[:], b2r[:], zeros[:], OP.add, OP.max
                )

            # ---- H1' (L3 fused): both tiles into one bank (row halves) ----
            ph = ppool.tile([128, L], f32, tag="ph", bufs=1, name="ph")
            for u in range(pglen):
                i = i0 + u
                for j in range(LANES):
                    nc.tensor.matmul(
                        ph[32 * j : 32 * j + 32, :],
                        m3[32 * j : 32 * j + 32, 32 * i : 32 * i + 32],
                        h2p[32 * j : 32 * j + 32, u * L : (u + 1) * L],
                        start=(u == 0), stop=(u == pglen - 1),
                        tile_position=(32 * j, 32 * j),
                        skip_group_check=True,
                    )
            hh = apool.tile([128, L], f16, tag="hh", name="hh")
            if nobias:
                nc.scalar.activation(hh[:], ph[:], AF.Relu)
            else:
                nc.scalar.activation(hh[:], ph[:], AF.Relu, bias=hb3[:, P : P + 1])

            # ---- H2: one merged matmul per pair, 4-tile group accumulation ----
            first_pair = (i0 % GRP) == 0
            last_pair = i0 + pglen >= G * GRP + glen
            if first_pair:
                py = ppool.tile([16, L], f32, tag="py", bufs=1, name="py")
            nc.tensor.matmul(
                py[:], wh2[:, 16 * P : 16 * P + 16], hh[:],
                start=first_pair, stop=last_pair,
                skip_group_check=True,
            )
            if last_pair:
                ysb = ypool.tile([16, L], f16, tag="ysb", name="ysb")
                if nobias:
                    nc.scalar.activation(ysb[:], py[:], AF.Copy)
                else:
                    nc.scalar.activation(
                        ysb[:], py[:], AF.Copy, bias=hb2[:, G : G + 1]
                    )
                nc.sync.dma_start(d_y.ap()[:, G * L : (G + 1) * L], ysb[:])
                del chunks[G]

    nc.compile()
    return nc


def _host_prep(x_cont, x_cate, t, emb, W1, b1, W2, b2, W3, b3, HW1, Hb1, HW2, Hb2):
    """Sort by head, shard, pad to single-head lanes; build weight tables."""
    import ml_dtypes

    f16 = np.float16
    f32 = np.float32
    f8 = ml_dtypes.float8_e4m3

    B = x_cont.shape[0]
    bs = B // N_CORES

    # ---- global sort by routing head (stable keeps shards contiguous) ----
    tt = t.reshape(-1).astype(np.int64)
    order = np.argsort(tt, kind="stable")

    # ---- per-core padded layout: every lane of L samples is single-head ----
    core_idx = []        # per core: int64 [bsp] global sample index (pads -> -1)
    core_lane_head = []  # per core: int64 [bsp//L] head id per lane
    for c in range(N_CORES):
        oc = order[c * bs : (c + 1) * bs]
        tc_ = tt[oc]
        idx_lanes = []
        head_lanes = []
        for n in range(NH):
            run = oc[tc_ == n]
            if run.size == 0:
                continue
            nlan = -(-run.size // L)
            padded = np.full(nlan * L, -1, np.int64)
            padded[: run.size] = run
            idx_lanes.append(padded)
            head_lanes.extend([n] * nlan)
        idx = np.concatenate(idx_lanes)
        core_idx.append(idx)
        core_lane_head.append(np.asarray(head_lanes, np.int64))

    # equalize + round lanes up to a tile multiple across all cores
    max_lanes = max(len(h) for h in core_lane_head)
    nlanes = -(-max_lanes // LANES) * LANES
    nt = nlanes // LANES
    ngroups = (nt + GRP - 1) // GRP
    npairs = (nt + 1) // 2
    for c in range(N_CORES):
        pad = nlanes - len(core_lane_head[c])
        if pad:
            core_idx[c] = np.concatenate(
                [core_idx[c], np.full(pad * L, -1, np.int64)]
            )
            core_lane_head[c] = np.concatenate(
                [core_lane_head[c], np.zeros(pad, np.int64)]
            )

    # ---- shared constants ----
    w1e8 = W1[CONT:].astype(f8)  # [128, 32], rows in (f*8+d) order
    w1cd = np.concatenate([W1[:CONT], W1[:CONT]], axis=0).astype(f16)  # [128, 32]

    w2bd = np.zeros((128, 128), f32)
    for j in range(LANES):
        w2bd[32 * j : 32 * j + 32, 32 * j : 32 * j + 32] = W2
    w2bd = w2bd.astype(f16)

    m3h = np.einsum("rk,nkh->nrh", W3, HW1)  # [NH, 32, 16] = W3 @ HW1[n]
    bias3h = b3 @ HW1.reshape(NH, RR, PH) + Hb1  # [NH, 16]

    # ---- embedding rows, features-major fp8: e8[f*8+d, b] ----
    flat_tab = emb.reshape(NF * VOCAB, EM).astype(f8)
    idx_flat = x_cate.astype(np.int64) + (np.arange(NF) * VOCAB)[None, :]
    e = flat_tab[idx_flat]  # [B, 16, 8] f8
    e8full = np.ascontiguousarray(e.reshape(-1, NF * EM).T)  # [128, B] f8
    xc16 = np.ascontiguousarray(x_cont.astype(f16).T)  # [64, B] f16

    nobias = not (
        np.any(b1) or np.any(b2) or np.any(b3) or np.any(Hb1) or np.any(Hb2)
    )

    in_maps = []
    for c in range(N_CORES):
        idx = core_idx[c]
        gidx = np.where(idx < 0, 0, idx)
        xcT = np.ascontiguousarray(xc16[:, gidx])
        e8 = np.ascontiguousarray(e8full[:, gidx])

        heads = core_lane_head[c].reshape(nt, LANES)
        # m3all: even tile [M3|0], odd tile [0|M3] (shared ph bank halves)
        # wh2p: one [128,16] block per PAIR; tile k of the pair reads hh rows
        # 32j+16*(k%2).. and writes py row 4*(k%GRP)+j.
        m3all = np.zeros((128, 32 * nt), f32)
        wh2p = np.zeros((128, 16 * npairs), f32)
        for i in range(nt):
            ro = 16 * (i % 2)
            for j in range(LANES):
                n = heads[i, j]
                m3all[32 * j : 32 * j + 32, 32 * i + ro : 32 * i + ro + 16] = m3h[n]
                wh2p[
                    32 * j + ro : 32 * j + ro + 16,
                    16 * (i // 2) + 4 * (i % GRP) + j,
                ] = HW2[n, :, 0]
        im = dict(
            xcT=xcT, e8=e8, w1e8=w1e8, w1cd=w1cd, w2bd=w2bd,
            m3all=m3all.astype(f16), wh2p=wh2p.astype(f16),
        )
        if not nobias:
            hb3 = np.zeros((128, npairs), f32)
            hb2 = np.zeros((16, ngroups), f32)
            for i in range(nt):
                ro = 16 * (i % 2)
                for j in range(LANES):
                    n = heads[i, j]
                    hb3[32 * j + ro : 32 * j + ro + 16, i // 2] = bias3h[n]
                    hb2[4 * (i % GRP) + j, i // GRP] = Hb2[n, 0]
            im.update(
                b1r=np.tile(b1, LANES).astype(f32)[:, None],
                b2r=np.tile(b2, LANES).astype(f32)[:, None],
                hb3=hb3, hb2=hb2,
            )
        in_maps.append(im)

    return in_maps, core_idx, nt, nobias


def kernel(**inputs):
    from concourse.bass_utils import run_bass_kernel_spmd

    x_cont = np.asarray(inputs["x_cont"], dtype=np.float32)
    x_cate = np.asarray(inputs["x_cate"])
    t = np.asarray(inputs["t"])
    emb = np.asarray(inputs["emb"], dtype=np.float32)
    args = [np.asarray(inputs[k], dtype=np.float32) for k in
            ("W1", "b1", "W2", "b2", "W3", "b3", "HW1", "Hb1", "HW2", "Hb2")]

    B = x_cont.shape[0]
    in_maps, core_idx, nt, nobias = _host_prep(x_cont, x_cate, t, emb, *args)

    key = (nt, nobias)
    if key not in _NC_CACHE:
        _NC_CACHE[key] = _build(nt, nobias=nobias)
    nc = _NC_CACHE[key]

    trace = os.environ.get("KERNEL_TRACE", "0") == "1"
    res = run_bass_kernel_spmd(nc, in_maps, core_ids=list(range(N_CORES)), trace=trace)
    global LAST
    LAST = res

    # ---- unsort: y[16, ngroups*L] -> padded order -> original order ----
    y = np.empty(B, np.float32)
    for c in range(N_CORES):
        ysb = np.asarray(res.results[c]["y"], dtype=np.float32)  # [16, ngroups*L]
        ngroups = ysb.shape[1] // L
        # row 4g+j, col G*L+k  ->  padded position ((G*GRP+g)*LANES+j)*L + k
        yp = ysb.reshape(GRP, LANES, ngroups, L).transpose(2, 0, 1, 3).reshape(-1)
        idx = core_idx[c]
        valid = idx >= 0
        y[idx[valid]] = yp[: idx.size][valid]
    return y


LAST = None


# revision 8
# speedup vs baseline: 1.0393x; 1.0393x over previous
"""Trainium2 Bass kernel for nn_CausalUnlabeled_2044404433206 (moe_routing).

Model per sample:
  e    = emb[f, x_cate[:, f]]                 (16 fields x 8 dims = 128 feats)
  x    = concat(x_cont[64], e[128])           -> 192
  h1   = relu(x @ W1 + b1)                    -> 32
  h2   = relu(h1 @ W2 + b2)                   -> 32
  r    = h2 @ W3 + b3                         -> 32
  hh   = relu(r @ HW1[t] + Hb1[t])            -> 16   (only the routed head)
  y    = hh @ HW2[t] + Hb2[t]

Design (v5):
  * Samples GLOBALLY SORTED by routing head t on the host (input
    marshalling, like the embedding gather), sharded contiguously across
    8 cores, padded so every 512-sample lane is single-headed.  Each
    lane uses only its own head's weights (baked into per-tile weight
    DATA m3all/wh2p since lhsT offsets are static), so head layers do
    1/8 the work and no one-hot select machinery is needed.
  * L3 fused into H1: r @ HW1[n] = h2 @ (W3 @ HW1[n]) = h2 @ M3[n],
    M3 precomputed host-side.  One less matmul stage + PSUM eviction.
  * Embedding features ship as fp8e4m3 (e-values ~0.05 meet e-weights
    ~0.05: fp8's ~2% element error lands at ~0.2% of h1).
  * DMA floor is ~2.9us per tile-pair (1MB @ ~358GB/s per-core HBM);
    everything else is structured to stay below it:
    - evictions run at FD=1024 over tile-PAIR psum tiles (p1p/p2p span
      2 banks; each tile's matmuls stay within one bank), split
      ACT: h1p+hh+y vs DVE: h2p -- each DVE op pays a full pipe DRAIN,
      so minimizing DVE op count matters more than balancing elements;
    - tile pairs share one H1' bank (even tile rows 32j..+16 via
      [M3|0] lhsT blocks, odd rows +16..+32 via [0|M3]), one hh
      eviction and ONE merged H2 matmul per pair (disjoint lhsT rows
      and output rows encode both tiles in one [128,16] block);
    - H2 accumulates a group of 4 tiles into one [16, L] bank.
  * PE HAM warm-up: the clock gate only opens (1.2 -> 2.4 GHz) after a
    ~3.4us continuously-busy window, and the steady-state loop never
    provides one -- so warmth must be established ONCE at the head and
    never lost (steady-state PE gaps stay far below the ~3.4us MID
    re-throttle window).  A 14-matmul back-to-back dummy burst starting
    right after the preamble bridges until the first chunk lands.
  * Head latency is DMA-issue-bound (~0.65us per HWDGE DIRECT2D on the
    Sync sequencer), so: xc ships pre-packed into partition halves
    0:64 / 64:128 (even/odd SBUF ports -> all 16 DMA engines, ONE issue
    per chunk), all f16 weight tables are packed into a single DRAM
    block (one issue), and chunk 0 is issued before everything else.
  * y stores stream out per group; output is f16 (y ~1e-3 scale, f16
    rounding ~5e-4 relative -- far under the 2e-2 gate).
"""

import os
import sys

sys.path.insert(0, "/opt/trn_rl_repo")

import numpy as np

B_FULL = 524288
CONT = 64
NF = 16  # categorical fields
VOCAB = 1000
EM = 8
LOW = EM * NF + CONT  # 192
RH = 32
RR = 32  # representation dim
PH = 16
NH = 8
N_CORES = 8
T = 2048  # samples per device tile
LANES = 4
L = T // LANES  # 512
GRP = 4  # tiles per H2 accumulation group / DMA chunk
HCH = GRP * T // 2  # xc packed-chunk width (4096)
NWARM = 14  # PE warm-up matmuls

_NC_CACHE = {}


def _build(nt, nobias=False):
    """Build + compile the per-core Bass program for nt tiles of T samples."""
    from contextlib import ExitStack

    import concourse.mybir as mybir
    import concourse.tile as tile
    from concourse import bacc

    f32 = mybir.dt.float32
    f16 = mybir.dt.float16
    f8 = mybir.dt.float8e4
    AF = mybir.ActivationFunctionType
    OP = mybir.AluOpType

    bs = nt * T
    ngroups = (nt + GRP - 1) // GRP
    npairs = (nt + 1) // 2
    CW = 32 + 128 + 32 * nt + 16 * npairs  # packed f16 const block width

    nc = bacc.Bacc(
        "TRN2",
        target_bir_lowering=False,
        debug=False,
        enable_asserts=False,
        num_devices=N_CORES,
    )

    # ---- DRAM I/O ----
    d_xcs = nc.dram_tensor("xcs", [128, ngroups * HCH], f16, kind="ExternalInput")
    d_e8 = nc.dram_tensor("e8", [128, bs], f8, kind="ExternalInput")
    d_w1e = nc.dram_tensor("w1e8", [128, RH], f8, kind="ExternalInput")
    d_cblk = nc.dram_tensor("cblk", [128, CW], f16, kind="ExternalInput")
    d_y = nc.dram_tensor("y", [16, ngroups * L], f16, kind="ExternalOutput")
    if not nobias:
        d_b1 = nc.dram_tensor("b1r", [128, 1], f32, kind="ExternalInput")
        d_b2 = nc.dram_tensor("b2r", [128, 1], f32, kind="ExternalInput")
        d_hb3 = nc.dram_tensor("hb3", [128, npairs], f32, kind="ExternalInput")
        d_hb2 = nc.dram_tensor("hb2", [16, ngroups], f32, kind="ExternalInput")

    with tile.TileContext(nc) as tc, ExitStack() as ctx:
        cpool = ctx.enter_context(tc.tile_pool(name="const", bufs=1))
        inpool = ctx.enter_context(tc.tile_pool(name="inp", bufs=3))
        apool = ctx.enter_context(tc.tile_pool(name="acts", bufs=3))
        ypool = ctx.enter_context(tc.tile_pool(name="yout", bufs=2))
        ppool = ctx.enter_context(tc.tile_pool(name="psum", bufs=1, space="PSUM"))

        # ---- input chunk loader: one xc + one e8 DMA per 4-tile group ----
        def load_chunk(G):
            glen = min(GRP, nt - G * GRP)
            clen = glen * T
            half = (glen + 1) // 2
            xcq = inpool.tile([128, HCH], f16, tag="xcq", name="xcq")
            nc.sync.dma_start(
                xcq[:, : half * T], d_xcs.ap()[:, G * HCH : G * HCH + half * T]
            )
            e8q = inpool.tile([128, GRP * T], f8, tag="e8q", name="e8q")
            nc.sync.dma_start(e8q[:, :clen], d_e8.ap()[:, G * GRP * T :][:, :clen])
            return xcq, e8q, half

        # chunk 0 first: it gates the warm-PE handoff to real work.
        chunks = {0: load_chunk(0)}

        cblk = cpool.tile([128, CW], f16, tag="cblk", name="cblk")
        nc.sync.dma_start(cblk[:], d_cblk.ap())
        w1cd = cblk[:, 0:32]
        w2bd = cblk[:, 32:160]
        m3 = cblk[:, 160 : 160 + 32 * nt]
        wh2 = cblk[:, 160 + 32 * nt : CW]
        w1e = cpool.tile([128, RH], f8, tag="w1e", name="w1e")
        nc.sync.dma_start(w1e[:], d_w1e.ap())

        if ngroups > 1:
            chunks[1] = load_chunk(1)

        if not nobias:
            def cload(dram, shape, tag):
                tl = cpool.tile(shape, f32, tag=tag, name=tag)
                nc.sync.dma_start(tl[:], dram.ap())
                return tl
            b1r = cload(d_b1, [128, 1], "b1r")
            b2r = cload(d_b2, [128, 1], "b2r")
            hb3 = cload(d_hb3, [128, npairs], "hb3")
            hb2 = cload(d_hb2, [16, ngroups], "hb2")
            zeros = cpool.tile([128, 2 * L], f16, tag="zeros", name="zeros")
            nc.vector.memset(zeros[:], 0.0)

        # ---- PE warm-up: back-to-back matmuls flip HAM to 2.4 GHz while
        # chunk 0 streams in.  M=16 keeps the LDWEIGHTS tiny.
        zwarm = cpool.tile([128, L], f16, tag="zwarm", name="zwarm")
        nc.vector.memset(zwarm[:], 0.0)
        wpy = ppool.tile([16, L], f32, tag="py", bufs=1, name="wpy")
        for _ in range(NWARM):
            nc.tensor.matmul(
                wpy[:], zwarm[:, :16], zwarm[:], start=True, stop=True,
                skip_group_check=True,
            )

        for P in range(npairs):
            i0 = 2 * P
            pglen = min(2, nt - i0)  # tiles in this pair (1 for odd tail)
            G = i0 // GRP
            glen = min(GRP, nt - G * GRP)

            if i0 % GRP == 0 and G + 2 < ngroups:
                chunks[G + 2] = load_chunk(G + 2)
            xcq, e8q, half = chunks[G]

            # ---- L1 + L2 per tile; pair shares 2-bank psum tiles ----
            p1p = ppool.tile([128, 2 * L], f32, tag="p1p", bufs=2, name="p1p")
            p2p = ppool.tile([128, 2 * L], f32, tag="p2p", bufs=1, name="p2p")
            for u in range(pglen):
                i = i0 + u
                g = i % GRP
                if g < half:
                    xrow, xcol = 0, g * T
                else:
                    xrow, xcol = CONT, (g - half) * T
                for j in range(LANES):
                    nc.tensor.matmul(
                        p1p[32 * j : 32 * j + 32, u * L : (u + 1) * L],
                        w1e[:],
                        e8q[:, (g * LANES + j) * L : (g * LANES + j + 1) * L],
                        start=True, stop=False, tile_position=(0, 32 * j),
                        skip_group_check=True,
                    )
                for j in range(LANES):
                    nc.tensor.matmul(
                        p1p[32 * j : 32 * j + 32, u * L : (u + 1) * L],
                        w1cd[xrow : xrow + CONT, :],
                        xcq[xrow : xrow + CONT, xcol + j * L : xcol + (j + 1) * L],
                        start=False, stop=True, tile_position=(xrow, 32 * j),
                        skip_group_check=True,
                    )
            h1p = apool.tile([128, 2 * L], f16, tag="h1p", name="h1p")
            if nobias:
                nc.scalar.activation(h1p[:], p1p[:], AF.Relu)
            else:
                nc.scalar.activation(h1p[:], p1p[:], AF.Relu, bias=b1r[:])

            for u in range(pglen):
                nc.tensor.matmul(
                    p2p[:, u * L : (u + 1) * L], w2bd[:],
                    h1p[:, u * L : (u + 1) * L],
                    start=True, stop=True, skip_group_check=True,
                )
            h2p = apool.tile([128, 2 * L], f16, tag="h2p", name="h2p")
            if nobias:
                nc.vector.tensor_scalar_max(h2p[:], p2p[:], 0.0)
            else:
                nc.vector.scalar_tensor_tensor(
                    h2p[:], p2p[:], b2r[:], zeros[:], OP.add, OP.max
                )

            # ---- H1' (L3 fused): both tiles into one bank (row halves) ----
            ph = ppool.tile([128, L], f32, tag="ph", bufs=1, name="ph")
            for u in range(pglen):
                i = i0 + u
                for j in range(LANES):
                    nc.tensor.matmul(
                        ph[32 * j : 32 * j + 32, :],
                        m3[32 * j : 32 * j + 32, 32 * i : 32 * i + 32],
                        h2p[32 * j : 32 * j + 32, u * L : (u + 1) * L],
                        start=(u == 0), stop=(u == pglen - 1),
                        tile_position=(32 * j, 32 * j),
                        skip_group_check=True,
                    )
            hh = apool.tile([128, L], f16, tag="hh", name="hh")
            if nobias:
                nc.scalar.activation(hh[:], ph[:], AF.Relu)
            else:
                nc.scalar.activation(hh[:], ph[:], AF.Relu, bias=hb3[:, P : P + 1])

            # ---- H2: one merged matmul per pair, 4-tile group accumulation ----
            first_pair = (i0 % GRP) == 0
            last_pair = i0 + pglen >= G * GRP + glen
            if first_pair:
                py = ppool.tile([16, L], f32, tag="py", bufs=1, name="py")
            nc.tensor.matmul(
                py[:], wh2[:, 16 * P : 16 * P + 16], hh[:],
                start=first_pair, stop=last_pair,
                skip_group_check=True,
            )
            if last_pair:
                ysb = ypool.tile([16, L], f16, tag="ysb", name="ysb")
                if nobias:
                    nc.scalar.activation(ysb[:], py[:], AF.Copy)
                else:
                    nc.scalar.activation(
                        ysb[:], py[:], AF.Copy, bias=hb2[:, G : G + 1]
                    )
                nc.sync.dma_start(d_y.ap()[:, G * L : (G + 1) * L], ysb[:])
                del chunks[G]

    nc.compile()
    return nc


def _host_prep(x_cont, x_cate, t, emb, W1, b1, W2, b2, W3, b3, HW1, Hb1, HW2, Hb2):
    """Sort by head, shard, pad to single-head lanes; build weight tables."""
    import ml_dtypes

    f16 = np.float16
    f32 = np.float32
    f8 = ml_dtypes.float8_e4m3

    B = x_cont.shape[0]
    bs = B // N_CORES

    # ---- global sort by routing head (stable keeps shards contiguous) ----
    tt = t.reshape(-1).astype(np.int64)
    order = np.argsort(tt, kind="stable")

    # ---- per-core padded layout: every lane of L samples is single-head ----
    core_idx = []        # per core: int64 [bsp] global sample index (pads -> -1)
    core_lane_head = []  # per core: int64 [bsp//L] head id per lane
    for c in range(N_CORES):
        oc = order[c * bs : (c + 1) * bs]
        tc_ = tt[oc]
        idx_lanes = []
        head_lanes = []
        for n in range(NH):
            run = oc[tc_ == n]
            if run.size == 0:
                continue
            nlan = -(-run.size // L)
            padded = np.full(nlan * L, -1, np.int64)
            padded[: run.size] = run
            idx_lanes.append(padded)
            head_lanes.extend([n] * nlan)
        idx = np.concatenate(idx_lanes)
        core_idx.append(idx)
        core_lane_head.append(np.asarray(head_lanes, np.int64))

    # equalize + round lanes up to a tile multiple across all cores
    max_lanes = max(len(h) for h in core_lane_head)
    nlanes = -(-max_lanes // LANES) * LANES
    nt = nlanes // LANES
    ngroups = (nt + GRP - 1) // GRP
    npairs = (nt + 1) // 2
    for c in range(N_CORES):
        pad = nlanes - len(core_lane_head[c])
        if pad:
            core_idx[c] = np.concatenate(
                [core_idx[c], np.full(pad * L, -1, np.int64)]
            )
            core_lane_head[c] = np.concatenate(
                [core_lane_head[c], np.zeros(pad, np.int64)]
            )

    # ---- shared constants ----
    w1e8 = W1[CONT:].astype(f8)  # [128, 32], rows in (f*8+d) order
    w1cd = np.concatenate([W1[:CONT], W1[:CONT]], axis=0)  # [128, 32]

    w2bd = np.zeros((128, 128), f32)
    for j in range(LANES):
        w2bd[32 * j : 32 * j + 32, 32 * j : 32 * j + 32] = W2

    m3h = np.einsum("rk,nkh->nrh", W3, HW1)  # [NH, 32, 16] = W3 @ HW1[n]
    bias3h = b3 @ HW1.reshape(NH, RR, PH) + Hb1  # [NH, 16]

    # ---- embedding rows, features-major fp8: e8[f*8+d, b] ----
    flat_tab = emb.reshape(NF * VOCAB, EM).astype(f8)
    idx_flat = x_cate.astype(np.int64) + (np.arange(NF) * VOCAB)[None, :]
    e = flat_tab[idx_flat]  # [B, 16, 8] f8
    e8full = np.ascontiguousarray(e.reshape(-1, NF * EM).T)  # [128, B] f8
    xc16 = np.ascontiguousarray(x_cont.astype(f16).T)  # [64, B] f16

    nobias = not (
        np.any(b1) or np.any(b2) or np.any(b3) or np.any(Hb1) or np.any(Hb2)
    )

    in_maps = []
    for c in range(N_CORES):
        idx = core_idx[c]
        gidx = np.where(idx < 0, 0, idx)
        xcT = xc16[:, gidx]  # [64, bsp]
        e8 = np.ascontiguousarray(e8full[:, gidx])

        # xc packed into partition halves per 4-tile chunk: chunk G's tiles
        # [0, half) go to rows 0:64 at cols G*HCH.., tiles [half, glen) to
        # rows 64:128 (matches the kernel's (g < half) addressing).
        xcs = np.zeros((128, ngroups * HCH), f16)
        for G in range(ngroups):
            glen = min(GRP, nt - G * GRP)
            half = (glen + 1) // 2
            c0 = G * GRP * T
            xcs[0:CONT, G * HCH : G * HCH + half * T] = (
                xcT[:, c0 : c0 + half * T]
            )
            if glen > half:
                xcs[CONT:128, G * HCH : G * HCH + (glen - half) * T] = (
                    xcT[:, c0 + half * T : c0 + glen * T]
                )

        heads = core_lane_head[c].reshape(nt, LANES)
        # m3all: even tile [M3|0], odd tile [0|M3] (shared ph bank halves)
        # wh2p: one [128,16] block per PAIR; tile k of the pair reads hh rows
        # 32j+16*(k%2).. and writes py row 4*(k%GRP)+j.
        m3all = np.zeros((128, 32 * nt), f32)
        wh2p = np.zeros((128, 16 * npairs), f32)
        for i in range(nt):
            ro = 16 * (i % 2)
            for j in range(LANES):
                n = heads[i, j]
                m3all[32 * j : 32 * j + 32, 32 * i + ro : 32 * i + ro + 16] = m3h[n]
                wh2p[
                    32 * j + ro : 32 * j + ro + 16,
                    16 * (i // 2) + 4 * (i % GRP) + j,
                ] = HW2[n, :, 0]
        cblk = np.concatenate([w1cd, w2bd, m3all, wh2p], axis=1).astype(f16)
        im = dict(xcs=xcs, e8=e8, w1e8=w1e8, cblk=cblk)
        if not nobias:
            hb3 = np.zeros((128, npairs), f32)
            hb2 = np.zeros((16, ngroups), f32)
            for i in range(nt):
                ro = 16 * (i % 2)
                for j in range(LANES):
                    n = heads[i, j]
                    hb3[32 * j + ro : 32 * j + ro + 16, i // 2] = bias3h[n]
                    hb2[4 * (i % GRP) + j, i // GRP] = Hb2[n, 0]
            im.update(
                b1r=np.tile(b1, LANES).astype(f32)[:, None],
                b2r=np.tile(b2, LANES).astype(f32)[:, None],
                hb3=hb3, hb2=hb2,
            )
        in_maps.append(im)

    return in_maps, core_idx, nt, nobias


def kernel(**inputs):
    from concourse.bass_utils import run_bass_kernel_spmd

    x_cont = np.asarray(inputs["x_cont"], dtype=np.float32)
    x_cate = np.asarray(inputs["x_cate"])
    t = np.asarray(inputs["t"])
    emb = np.asarray(inputs["emb"], dtype=np.float32)
    args = [np.asarray(inputs[k], dtype=np.float32) for k in
            ("W1", "b1", "W2", "b2", "W3", "b3", "HW1", "Hb1", "HW2", "Hb2")]

    B = x_cont.shape[0]
    in_maps, core_idx, nt, nobias = _host_prep(x_cont, x_cate, t, emb, *args)

    key = (nt, nobias)
    if key not in _NC_CACHE:
        _NC_CACHE[key] = _build(nt, nobias=nobias)
    nc = _NC_CACHE[key]

    trace = os.environ.get("KERNEL_TRACE", "0") == "1"
    res = run_bass_kernel_spmd(nc, in_maps, core_ids=list(range(N_CORES)), trace=trace)
    global LAST
    LAST = res

    # ---- unsort: y[16, ngroups*L] -> padded order -> original order ----
    y = np.empty(B, np.float32)
    for c in range(N_CORES):
        ysb = np.asarray(res.results[c]["y"], dtype=np.float32)  # [16, ngroups*L]
        ngroups = ysb.shape[1] // L
        # row 4g+j, col G*L+k  ->  padded position ((G*GRP+g)*LANES+j)*L + k
        yp = ysb.reshape(GRP, LANES, ngroups, L).transpose(2, 0, 1, 3).reshape(-1)
        idx = core_idx[c]
        valid = idx >= 0
        y[idx[valid]] = yp[: idx.size][valid]
    return y


LAST = None
